# revision 18
# baseline (speedup 1.0000x reference)
"""MultiHeadCrossAttentionFusion kernel for TRN2 (8 NeuronCores, data-parallel over batch).

v2 design (per core, batch shard BS=1024):
  Phase A: x -> xT (PE transpose, batched psum->sbuf copies), QKV computed
           TRANSPOSED (lhsT=W tiles, rhs=xT) -> qkvT [128, 24, 1024] bf16,
           SBUF-resident. Bias (if any) added per-partition.
  Phase B: per 128-row b-tile and branch: pack q/k/v into
           [73, 16j x 128] tiles (col = j*128 + par*64 + hp*8 + b8) with 9
           constant rows appended that realize the block-diagonal softmax
           mask INSIDE the score matmul (rank-9 decomposition of the mask).
           Per 4-j group: batched vp transposes, one exp, caT = vp^T @ eT
           (with a ones column producing the softmax denominator as row 64),
           batched reciprocal, z-broadcast via K=1 outer-product matmul,
           fused scale+scatter into caT_all [128, idx, hp, b].
  Stats:   LN mean/var via ones-matmuls over caT_all, batched x4.
  Phase C: projection matmuls (LN folded into host-precomputed weights,
           mean handled as a rank-1 update), r-scale + residual, output.
"""
import sys
sys.path.insert(0, "/opt/trn_rl_repo")
import numpy as np
import ml_dtypes
from contextlib import ExitStack

import concourse.bass as bass
from concourse import bacc as _bacc
import concourse.mybir as mybir
from concourse.tile import TileContext
from concourse.bass_utils import run_bass_kernel_spmd

B, CD, HID, H, D = 8192, 2048, 1024, 16, 64
NCORES = 8
BS = B // NCORES          # 1024 rows per core
NB = BS // 128            # 8 b-tiles
KT = CD // 128            # 16 k-tiles for qkv matmul
NT = (3 * HID) // 128     # 24 qkvT n-tiles
CT = HID // 128           # 8 proj k-tiles
NCHP = CD // 512          # 4 proj n-chunks
EPS = 1e-5
F32 = mybir.dt.float32
BF16 = mybir.dt.bfloat16
AL = mybir.AluOpType
AF = mybir.ActivationFunctionType


def build_nc(with_bias=True, linearize=False):
    nc = _bacc.Bacc()
    dp = nc.declare_dram_parameter
    x_c = dp("x_c", [BS, CD], F32, isOutput=False)
    x_m = dp("x_m", [BS, CD], F32, isOutput=False)
    Wq_c = dp("Wq_c", [CD, 3 * HID], BF16, isOutput=False)
    Wq_m = dp("Wq_m", [CD, 3 * HID], BF16, isOutput=False)
    bq_c = dp("bq_c", [3 * HID, 1], F32, isOutput=False)
    bq_m = dp("bq_m", [3 * HID, 1], F32, isOutput=False)
    Wg_c = dp("Wg_c", [HID, CD], BF16, isOutput=False)   # g-folded, permuted
    Wg_m = dp("Wg_m", [HID, CD], BF16, isOutput=False)
    v_c = dp("v_c", [1, CD], F32, isOutput=False)        # be@Wp + b_proj
    v_m = dp("v_m", [1, CD], F32, isOutput=False)
    un_c = dp("un_c", [1, CD], F32, isOutput=False)      # -(gW).sum(0)
    un_m = dp("un_m", [1, CD], F32, isOutput=False)
    mq9 = dp("mq9", [9, 2048], BF16, isOutput=False)     # mask rows (q side)
    mk9 = dp("mk9", [9, 2048], BF16, isOutput=False)     # mask rows (k side)
    identb = dp("identb", [128, 128], BF16, isOutput=False)
    ones_bf = dp("ones_bf", [128, 1], BF16, isOutput=False)
    ones1f = dp("ones1f", [1, 64], F32, isOutput=False)
    onesr_f = dp("onesr_f", [1, 128], F32, isOutput=False)
    out_c = dp("out_c", [BS, CD], F32, isOutput=True)
    out_m = dp("out_m", [BS, CD], F32, isOutput=True)

    with TileContext(nc, linearize=linearize) as tc, ExitStack() as ctx:
        consts = ctx.enter_context(tc.tile_pool(name="consts", bufs=1))
        keep = ctx.enter_context(tc.tile_pool(name="keep", bufs=1))

        sb_id = consts.tile([128, 128], BF16)
        nc.sync.dma_start(sb_id, identb[:, :])
        sb_ones = consts.tile([128, 1], BF16)
        nc.sync.dma_start(sb_ones, ones_bf[:, :])
        sb_o1f = consts.tile([1, 64], F32)
        nc.sync.dma_start(sb_o1f, ones1f[:, :])
        ung = {}
        for t, un in (("c", un_c), ("m", un_m)):
            ung[t] = consts.tile([1, CD], F32, name=f"ung_{t}", tag=f"ung_{t}")
            nc.sync.dma_start(ung[t], un[:, :])
        sb_bq = {}
        if with_bias:
            for t, bq in (("c", bq_c), ("m", bq_m)):
                sb_bq[t] = consts.tile([128, NT, 1], F32, name=f"bq_{t}",
                                       tag=f"bq_{t}")
                nc.sync.dma_start(
                    sb_bq[t], bq[:, :].rearrange("(t p) o -> p t o", p=128))

        pcat = ctx.enter_context(tc.tile_pool(name="pcat", bufs=1))
        caT_all = pcat.tile([128, 16, CT, 128], BF16, name="caT_all",
                            tag="caT_all")
        r_all = pcat.tile([128, 16], F32, name="r_all", tag="r_all")
        mu_all = pcat.tile([1, 16 * 128], F32, name="mu_all", tag="mu_all")

        pqk_cm = tc.tile_pool(name="pqk", bufs=1)
        pqk = pqk_cm.__enter__()
        # qkvT[t][p, nt, b] = (x_t @ Wq_t)^T tile layout; n = nt*128 + p
        qkvT = {
            "c": pqk.tile([128, NT, BS], BF16, name="qkvT_c", tag="qkvT_c"),
            "m": pqk.tile([128, NT, BS], BF16, name="qkvT_m", tag="qkvT_m"),
        }

        # ---- Phase A: xT build + transposed QKV matmuls ----
        with tc.tile_pool(name="pxt", bufs=1) as pxt, \
             tc.tile_pool(name="pin", bufs=2) as pin, \
             tc.tile_pool(name="pw", bufs=2) as pw, \
             tc.tile_pool(name="psTA", bufs=2, space="PSUM") as psTA, \
             tc.tile_pool(name="psA", bufs=2, space="PSUM") as psA:
            for ti, (t, xin, Wt) in enumerate(
                    (("c", x_c, Wq_c), ("m", x_m, Wq_m))):
                xT = pxt.tile([128, KT, BS], BF16, tag="xT")
                for bt in range(NB):
                    for qf in range(4):
                        xn = pin.tile([128, CD // 4], F32, tag="xn")
                        nc.sync.dma_start(
                            xn, xin[bt * 128:(bt + 1) * 128,
                                    qf * 512:(qf + 1) * 512])
                        xb = pin.tile([128, CD // 4], BF16, tag="xb")
                        nc.vector.tensor_copy(out=xb, in_=xn)
                        pt = psTA.tile([128, 512], BF16, tag="ptx")
                        for kk in range(4):
                            nc.tensor.transpose(
                                pt[:, kk * 128:(kk + 1) * 128],
                                xb[:, kk * 128:(kk + 1) * 128], sb_id)
                        dst = xT[:, qf * 4:(qf + 1) * 4,
                                 bt * 128:(bt + 1) * 128]
                        src = pt[:, :].rearrange("p (k b) -> p k b", k=4)
                        if (bt + qf) % 2 == 0:
                            nc.vector.tensor_copy(out=dst, in_=src)
                        else:
                            nc.scalar.copy(out=dst, in_=src)
                for nt in range(NT):
                    wst = pw.tile([128, KT, 128], BF16, tag="wst")
                    nc.sync.dma_start(
                        wst, Wt[:, nt * 128:(nt + 1) * 128].rearrange(
                            "(kt p) n -> p kt n", p=128))
                    for bc in range(2):
                        px = psA.tile([128, 512], F32, tag="px")
                        for kt in range(KT):
                            nc.tensor.matmul(
                                px, lhsT=wst[:, kt, :],
                                rhs=xT[:, kt, bc * 512:(bc + 1) * 512],
                                start=(kt == 0), stop=(kt == KT - 1))
                        dst = qkvT[t][:, nt, bc * 512:(bc + 1) * 512]
                        if with_bias:
                            nc.vector.tensor_scalar(
                                out=dst, in0=px,
                                scalar1=sb_bq[t][:, nt, 0:1],
                                scalar2=None, op0=AL.add)
                        elif (nt + bc) % 2 == 0:
                            nc.vector.tensor_copy(out=dst, in_=px)
                        else:
                            nc.scalar.copy(out=dst, in_=px)

        # ---- Phase B: attention ----
        ppk_cm = tc.tile_pool(name="ppk", bufs=1)
        ppk = ppk_cm.__enter__()
        qpk = []
        kpk = []
        vpk = []
        for i in range(2):
            qp = ppk.tile([73, 2048], BF16, name=f"qpk{i}", tag=f"qpk{i}")
            nc.sync.dma_start(qp[64:73, :], mq9[:, :])
            qpk.append(qp)
            kp = ppk.tile([73, 2048], BF16, name=f"kpk{i}", tag=f"kpk{i}")
            nc.sync.dma_start(kp[64:73, :], mk9[:, :])
            kpk.append(kp)
            vpk.append(ppk.tile([64, 2048], BF16, name=f"vpk{i}",
                                tag=f"vpk{i}"))
        vp_all = ppk.tile([128, 16, 65], BF16, name="vp_all", tag="vp_all")
        for j in range(16):
            nc.scalar.copy(out=vp_all[:, j, 64:65], in_=sb_ones)

        with tc.tile_pool(name="peT", bufs=3) as peT, \
             tc.tile_pool(name="prz", bufs=2) as prz, \
             tc.tile_pool(name="psV", bufs=2, space="PSUM") as psV, \
             tc.tile_pool(name="psS", bufs=2, space="PSUM") as psS, \
             tc.tile_pool(name="psC", bufs=2, space="PSUM") as psC, \
             tc.tile_pool(name="psR", bufs=2, space="PSUM") as psR:
            for bt in range(NB):
                for bri, (qs, ks) in enumerate((("c", "m"), ("m", "c"))):
                    idx = bt * 2 + bri
                    ib = idx % 2
                    qp, kp, vp_k = qpk[ib], kpk[ib], vpk[ib]
                    # pack q/k/v: col = j*128 + par*64 + hp*8 + b8
                    for par in range(2):
                        for pki, (pk, tsrc, base) in enumerate(
                                ((qp, qs, 0), (kp, ks, 8), (vp_k, ks, 16))):
                            src = qkvT[tsrc][
                                par * 64:(par + 1) * 64, base:base + 8,
                                bt * 128:(bt + 1) * 128].rearrange(
                                    "p hp (j b) -> p hp j b", j=16)
                            dst = pk[0:64, :].rearrange(
                                "p (j q hp b) -> p j q hp b",
                                j=16, q=2, hp=8)[:, :, par, :, :].transpose(
                                    [0, 2, 1, 3])
                            if pki == 0:
                                nc.vector.tensor_copy(out=dst, in_=src)
                            elif pki == 1:
                                nc.scalar.copy(out=dst, in_=src)
                            else:
                                nc.gpsimd.tensor_copy(out=dst, in_=src)
                    for jg in range(4):
                        psV4 = psV.tile([128, 256], BF16, tag="v4")
                        for jj in range(4):
                            j = jg * 4 + jj
                            nc.tensor.transpose(
                                psV4[:, jj * 64:(jj + 1) * 64],
                                vp_k[0:64, j * 128:(j + 1) * 128],
                                sb_id[0:64, 0:64])
                        nc.scalar.copy(
                            out=vp_all[:, jg * 4:(jg + 1) * 4, 0:64],
                            in_=psV4[:, :].rearrange("p (j d) -> p j d", j=4))
                        psS4 = psS.tile([128, 512], F32, tag="s4")
                        for jj in range(4):
                            j = jg * 4 + jj
                            nc.tensor.matmul(
                                psS4[:, jj * 128:(jj + 1) * 128],
                                lhsT=kp[:, j * 128:(j + 1) * 128],
                                rhs=qp[:, j * 128:(j + 1) * 128],
                                start=True, stop=True)
                        eT4 = peT.tile([128, 512], BF16, tag="e4")
                        nc.scalar.activation(eT4, psS4, AF.Exp, scale=0.125)
                        psC4 = psC.tile([65, 512], F32, tag="c4")
                        for jj in range(4):
                            j = jg * 4 + jj
                            nc.tensor.matmul(
                                psC4[:, jj * 128:(jj + 1) * 128],
                                lhsT=vp_all[:, j, :],
                                rhs=eT4[:, jj * 128:(jj + 1) * 128],
                                start=True, stop=True)
                        zs = prz.tile([1, 512], F32, tag="rz")
                        nc.scalar.copy(out=zs, in_=psC4[64:65, :])
                        rep_ps = psR.tile([64, 512], F32, tag="rep")
                        nc.tensor.matmul(rep_ps, lhsT=sb_o1f, rhs=zs,
                                         start=True, stop=True)
                        rep_sb = peT.tile([64, 512], F32, tag="repsb")
                        nc.vector.reciprocal_approx_fast(out=rep_sb,
                                                         in_=rep_ps)
                        for par in range(2):
                            in0 = psC4[0:64, :].rearrange(
                                "p (j q hp b) -> p j q hp b",
                                j=4, q=2, hp=8)[:, :, par, :, :]
                            in1 = rep_sb[:, :].rearrange(
                                "p (j q hp b) -> p j q hp b",
                                j=4, q=2, hp=8)[:, :, par, :, :]
                            dst = caT_all[
                                par * 64:(par + 1) * 64, idx, 0:8,
                                jg * 32:(jg + 1) * 32].rearrange(
                                    "p hp (j b) -> p hp j b", j=4).transpose(
                                        [0, 2, 1, 3])
                            nc.vector.tensor_tensor(
                                out=dst, in0=in0, in1=in1, op=AL.mult)

        ppk_cm.__exit__(None, None, None)
        pqk_cm.__exit__(None, None, None)

        # ---- LN stats (batched x4) ----
        with tc.tile_pool(name="pstt", bufs=2) as pstt, \
             tc.tile_pool(name="psqq", bufs=1) as psqq, \
             tc.tile_pool(name="psSt", bufs=2, space="PSUM") as psSt, \
             tc.tile_pool(name="psRT", bufs=2, space="PSUM") as psRT:
            sq = psqq.tile([128, 16, CT, 128], BF16, tag="sq")
            nc.vector.tensor_tensor(out=sq, in0=caT_all, in1=caT_all,
                                    op=AL.mult)
            for g4 in range(4):
                mps = psSt.tile([1, 512], F32, tag="m4")
                sps = psSt.tile([1, 512], F32, tag="sg4")
                for i in range(4):
                    idx = g4 * 4 + i
                    for ct in range(CT):
                        nc.tensor.matmul(
                            mps[0:1, i * 128:(i + 1) * 128], lhsT=sb_ones,
                            rhs=caT_all[:, idx, ct, :],
                            start=(ct == 0), stop=(ct == CT - 1))
                    for ct in range(CT):
                        nc.tensor.matmul(
                            sps[0:1, i * 128:(i + 1) * 128], lhsT=sb_ones,
                            rhs=sq[:, idx, ct, :],
                            start=(ct == 0), stop=(ct == CT - 1))
                murow = pstt.tile([1, 512], F32, tag="murow")
                nc.vector.tensor_scalar(
                    out=murow, in0=mps, scalar1=1.0 / HID, scalar2=None,
                    op0=AL.mult)
                nc.vector.tensor_copy(
                    out=mu_all[:, g4 * 512:(g4 + 1) * 512], in_=murow)
                mu2 = pstt.tile([1, 512], F32, tag="mu2")
                nc.vector.tensor_tensor(out=mu2, in0=murow, in1=murow,
                                        op=AL.mult)
                vvr = pstt.tile([1, 512], F32, tag="vvr")
                nc.vector.tensor_scalar(
                    out=vvr, in0=sps, scalar1=1.0 / HID, scalar2=EPS,
                    op0=AL.mult, op1=AL.add)
                vv2 = pstt.tile([1, 512], F32, tag="vv2")
                nc.vector.tensor_tensor(out=vv2, in0=vvr, in1=mu2,
                                        op=AL.subtract)
                rt = psRT.tile([128, 4], F32, tag="rt")
                for i in range(4):
                    nc.tensor.transpose(
                        rt[:, i:i + 1], vv2[0:1, i * 128:(i + 1) * 128],
                        sb_o1f[0:1, 0:1])
                sdc = pstt.tile([128, 4], F32, tag="sdc")
                nc.scalar.activation(sdc, rt, AF.Sqrt)
                nc.vector.reciprocal(r_all[:, g4 * 4:(g4 + 1) * 4], sdc)

        # ---- Phase C: projection + residual ----
        with tc.tile_pool(name="pwg", bufs=2) as pwg, \
             tc.tile_pool(name="pxr", bufs=4) as pxr, \
             tc.tile_pool(name="pot", bufs=4) as pot, \
             tc.tile_pool(name="psP", bufs=2, space="PSUM") as psP:
            sb_v = {}
            if with_bias:
                sb_or = consts.tile([1, 128], F32)
                nc.sync.dma_start(sb_or, onesr_f[:, :])
                for t, vv in (("c", v_c), ("m", v_m)):
                    row = consts.tile([1, CD], F32, name=f"vr_{t}",
                                      tag=f"vr_{t}")
                    nc.sync.dma_start(row, vv[:, :])
                    sb_v[t] = consts.tile([128, CD], F32, name=f"vb_{t}",
                                          tag=f"vb_{t}")
                    for ch in range(NCHP):
                        vps = psP.tile([128, 512], F32, tag="px")
                        nc.tensor.matmul(
                            vps, lhsT=sb_or,
                            rhs=row[0:1, ch * 512:(ch + 1) * 512],
                            start=True, stop=True)
                        nc.scalar.copy(
                            out=sb_v[t][:, ch * 512:(ch + 1) * 512], in_=vps)
            for nch in range(NCHP):
                wg = {}
                for t, Wgt in (("c", Wg_c), ("m", Wg_m)):
                    w = pwg.tile([128, CT, 512], BF16, tag=f"wg_{t}")
                    nc.sync.dma_start(
                        w, Wgt[:, nch * 512:(nch + 1) * 512].rearrange(
                            "(ct p) n -> p ct n", p=128))
                    wg[t] = w
                for bt in range(NB):
                    for bri, qs in enumerate(("c", "m")):
                        idx = bt * 2 + bri
                        xin = x_c if qs == "c" else x_m
                        xres = pxr.tile([128, 512], F32, tag="xr")
                        nc.sync.dma_start(
                            xres, xin[bt * 128:(bt + 1) * 128,
                                      nch * 512:(nch + 1) * 512])
                        px = psP.tile([128, 512], F32, tag="px")
                        for ct in range(CT):
                            nc.tensor.matmul(
                                px, lhsT=caT_all[:, idx, ct, :],
                                rhs=wg[qs][:, ct, :],
                                start=(ct == 0), stop=False)
                        nc.tensor.matmul(
                            px, lhsT=mu_all[:, idx * 128:(idx + 1) * 128],
                            rhs=ung[qs][:, nch * 512:(nch + 1) * 512],
                            start=False, stop=True)
                        t1 = pot.tile([128, 512], F32, tag="t1")
                        nc.scalar.activation(t1, px, AF.Copy,
                                             scale=r_all[:, idx:idx + 1])
                        ot = pot.tile([128, 512], F32, tag="ot")
                        eng = nc.vector if idx % 2 == 0 else nc.gpsimd
                        eng.tensor_tensor(out=ot, in0=t1, in1=xres, op=AL.add)
                        if with_bias:
                            o2 = pot.tile([128, 512], F32, tag="o2")
                            nc.vector.tensor_tensor(
                                out=o2, in0=ot,
                                in1=sb_v[qs][:, nch * 512:(nch + 1) * 512],
                                op=AL.add)
                            ot = o2
                        outt = out_c if qs == "c" else out_m
                        nc.sync.dma_start(
                            outt[bt * 128:(bt + 1) * 128,
                                 nch * 512:(nch + 1) * 512], ot)
    return nc


_NC = {}


def _get_nc(with_bias):
    if with_bias not in _NC:
        nc = build_nc(with_bias=with_bias)
        if not nc.is_finalized():
            nc.finalize()
        _NC[with_bias] = nc
    return _NC[with_bias]


def _host_prep(inputs):
    f32 = np.float32
    bf = ml_dtypes.bfloat16
    g = {k: np.asarray(v) for k, v in inputs.items()}
    # device ca layout: c_dev = hp*128 + par*64 + d, head h = 2*hp + par
    cdev = np.arange(HID)
    hp_t = cdev // 128
    par_t = (cdev % 128) // 64
    d_t = cdev % 64
    h_t = 2 * hp_t + par_t
    pr = d_t * H + h_t                   # ref col for each c_dev row
    consts = {}
    for t, (Wp, bp, g1, be1) in (
            ("c", ("W_cproj", "b_cproj", "g1", "be1")),
            ("m", ("W_mproj", "b_mproj", "g2", "be2"))):
        W = np.asarray(g[Wp], f32)[pr, :]          # [HID, CD] permuted
        g1d = np.asarray(g[g1], f32)[pr]
        be1d = np.asarray(g[be1], f32)[pr]
        consts[f"Wg_{t}"] = np.ascontiguousarray(
            (g1d[:, None] * W)).astype(bf)
        consts[f"v_{t}"] = (be1d @ W + np.asarray(g[bp], f32)).reshape(1, CD)\
            .astype(f32)
        consts[f"un_{t}"] = (-(g1d[:, None] * W).sum(0)).reshape(1, CD)\
            .astype(f32)
    consts["Wq_c"] = np.asarray(g["W_cqkv"], f32).astype(bf)
    consts["Wq_m"] = np.asarray(g["W_mqkv"], f32).astype(bf)
    consts["bq_c"] = np.asarray(g["b_cqkv"], f32).reshape(3 * HID, 1)
    consts["bq_m"] = np.asarray(g["b_mqkv"], f32).reshape(3 * HID, 1)
    # mask rows: col within a 128-col j-tile: c = par*64 + hp*8 + b8
    s8 = np.float32(bf(np.sqrt(800.0)))
    c = np.arange(128)
    b8 = c % 8
    mq = np.zeros((9, 128), f32)
    mk = np.zeros((9, 128), f32)
    for r in range(8):
        mq[r] = np.where(b8 == r, s8, 0.0)
        mk[r] = np.where(b8 == r, s8, 0.0)
    mq[8] = -s8
    mk[8] = s8
    consts["mq9"] = np.tile(mq, (1, 16)).astype(bf)
    consts["mk9"] = np.tile(mk, (1, 16)).astype(bf)
    consts["identb"] = np.eye(128).astype(bf)
    consts["ones_bf"] = np.ones((128, 1)).astype(bf)
    consts["ones1f"] = np.ones((1, 64)).astype(f32)
    consts["onesr_f"] = np.ones((1, 128)).astype(f32)
    return g, consts


def kernel(**inputs):
    g, consts = _host_prep(inputs)
    xc = np.ascontiguousarray(np.asarray(g["cnn_out"], np.float32))
    xm = np.ascontiguousarray(np.asarray(g["mlp_out"], np.float32))
    wb = (np.abs(consts["bq_c"]).max() > 0 or np.abs(consts["bq_m"]).max() > 0
          or np.abs(consts["v_c"]).max() > 0 or np.abs(consts["v_m"]).max() > 0)
    nc = _get_nc(bool(wb))
    in_maps = []
    for i in range(NCORES):
        m = dict(consts)
        m["x_c"] = xc[i * BS:(i + 1) * BS]
        m["x_m"] = xm[i * BS:(i + 1) * BS]
        in_maps.append(m)
    res = run_bass_kernel_spmd(nc, in_maps, list(range(NCORES))).results
    out_c = np.concatenate([np.asarray(res[i]["out_c"]) for i in range(NCORES)], 0)
    out_m = np.concatenate([np.asarray(res[i]["out_m"]) for i in range(NCORES)], 0)
    return (out_c.astype(np.float32), out_m.astype(np.float32))


# revision 19
# speedup vs baseline: 1.1986x; 1.1986x over previous
"""MultiHeadCrossAttentionFusion kernel for TRN2 (8 NeuronCores, data-parallel over batch).

v2 design (per core, batch shard BS=1024):
  Phase A: x -> xT (PE transpose, batched psum->sbuf copies), QKV computed
           TRANSPOSED (lhsT=W tiles, rhs=xT) -> qkvT [128, 24, 1024] bf16,
           SBUF-resident. Bias (if any) added per-partition.
  Phase B: per 128-row b-tile and branch: pack q/k/v into
           [73, 16j x 128] tiles (col = j*128 + par*64 + hp*8 + b8) with 9
           constant rows appended that realize the block-diagonal softmax
           mask INSIDE the score matmul (rank-9 decomposition of the mask).
           Per 4-j group: batched vp transposes, one exp, caT = vp^T @ eT
           (with a ones column producing the softmax denominator as row 64),
           batched reciprocal, z-broadcast via K=1 outer-product matmul,
           fused scale+scatter into caT_all [128, idx, hp, b].
  Stats:   LN mean/var via ones-matmuls over caT_all, batched x4.
  Phase C: projection matmuls (LN folded into host-precomputed weights,
           mean handled as a rank-1 update), r-scale + residual, output.
"""
import sys
sys.path.insert(0, "/opt/trn_rl_repo")
import numpy as np
import ml_dtypes
from contextlib import ExitStack

import concourse.bass as bass
from concourse import bacc as _bacc
import concourse.mybir as mybir
from concourse.tile import TileContext
from concourse.bass_utils import run_bass_kernel_spmd

B, CD, HID, H, D = 8192, 2048, 1024, 16, 64
NCORES = 8
BS = B // NCORES          # 1024 rows per core
NB = BS // 128            # 8 b-tiles
KT = CD // 128            # 16 k-tiles for qkv matmul
NT = (3 * HID) // 128     # 24 qkvT n-tiles
CT = HID // 128           # 8 proj k-tiles
NCHP = CD // 512          # 4 proj n-chunks
EPS = 1e-5
F32 = mybir.dt.float32
BF16 = mybir.dt.bfloat16
AL = mybir.AluOpType
AF = mybir.ActivationFunctionType


def build_nc(with_bias=True, linearize=False):
    nc = _bacc.Bacc()
    dp = nc.declare_dram_parameter
    x_c = dp("x_c", [BS, CD], F32, isOutput=False)
    x_m = dp("x_m", [BS, CD], F32, isOutput=False)
    Wq_c = dp("Wq_c", [CD, 3 * HID], BF16, isOutput=False)
    Wq_m = dp("Wq_m", [CD, 3 * HID], BF16, isOutput=False)
    bq_c = dp("bq_c", [3 * HID, 1], F32, isOutput=False)
    bq_m = dp("bq_m", [3 * HID, 1], F32, isOutput=False)
    Wg_c = dp("Wg_c", [HID, CD], BF16, isOutput=False)   # g-folded, permuted
    Wg_m = dp("Wg_m", [HID, CD], BF16, isOutput=False)
    v_c = dp("v_c", [1, CD], F32, isOutput=False)        # be@Wp + b_proj
    v_m = dp("v_m", [1, CD], F32, isOutput=False)
    un_c = dp("un_c", [1, CD], F32, isOutput=False)      # -(gW).sum(0)
    un_m = dp("un_m", [1, CD], F32, isOutput=False)
    mq9 = dp("mq9", [9, 2048], BF16, isOutput=False)     # mask rows (q side)
    mk9 = dp("mk9", [9, 2048], BF16, isOutput=False)     # mask rows (k side)
    identb = dp("identb", [128, 128], BF16, isOutput=False)
    ones_bf = dp("ones_bf", [128, 1], BF16, isOutput=False)
    ones1f = dp("ones1f", [1, 64], F32, isOutput=False)
    onesr_f = dp("onesr_f", [1, 128], F32, isOutput=False)
    out_c = dp("out_c", [BS, CD], F32, isOutput=True)
    out_m = dp("out_m", [BS, CD], F32, isOutput=True)

    with TileContext(nc, linearize=linearize) as tc, ExitStack() as ctx:
        consts = ctx.enter_context(tc.tile_pool(name="consts", bufs=1))
        keep = ctx.enter_context(tc.tile_pool(name="keep", bufs=1))

        sb_id = consts.tile([128, 128], BF16)
        nc.sync.dma_start(sb_id, identb[:, :])
        sb_ones = consts.tile([128, 1], BF16)
        nc.sync.dma_start(sb_ones, ones_bf[:, :])
        sb_o1f = consts.tile([1, 64], F32)
        nc.sync.dma_start(sb_o1f, ones1f[:, :])
        ung = {}
        for t, un in (("c", un_c), ("m", un_m)):
            ung[t] = consts.tile([1, CD], F32, name=f"ung_{t}", tag=f"ung_{t}")
            nc.sync.dma_start(ung[t], un[:, :])
        sb_bq = {}
        if with_bias:
            for t, bq in (("c", bq_c), ("m", bq_m)):
                sb_bq[t] = consts.tile([128, NT, 1], F32, name=f"bq_{t}",
                                       tag=f"bq_{t}")
                nc.sync.dma_start(
                    sb_bq[t], bq[:, :].rearrange("(t p) o -> p t o", p=128))

        pcat = ctx.enter_context(tc.tile_pool(name="pcat", bufs=1))
        caT_all = pcat.tile([128, 16, CT, 128], BF16, name="caT_all",
                            tag="caT_all")
        r_all = pcat.tile([128, 16], F32, name="r_all", tag="r_all")
        mu_all = pcat.tile([1, 16 * 128], F32, name="mu_all", tag="mu_all")

        pqk_cm = tc.tile_pool(name="pqk", bufs=1)
        pqk = pqk_cm.__enter__()
        # qkvT[t][p, nt, b] = (x_t @ Wq_t)^T tile layout; n = nt*128 + p
        qkvT = {
            "c": pqk.tile([128, NT, BS], BF16, name="qkvT_c", tag="qkvT_c"),
            "m": pqk.tile([128, NT, BS], BF16, name="qkvT_m", tag="qkvT_m"),
        }

        # ---- Phase A: xT build + transposed QKV matmuls ----
        with tc.tile_pool(name="pxt", bufs=1) as pxt, \
             tc.tile_pool(name="pin", bufs=2) as pin, \
             tc.tile_pool(name="pw", bufs=2) as pw, \
             tc.tile_pool(name="psTA", bufs=2, space="PSUM") as psTA, \
             tc.tile_pool(name="psA", bufs=2, space="PSUM") as psA:
            for ti, (t, xin, Wt) in enumerate(
                    (("c", x_c, Wq_c), ("m", x_m, Wq_m))):
                xT = pxt.tile([128, KT, BS], BF16, tag="xT")
                for bt in range(NB):
                    for qf in range(4):
                        xn = pin.tile([128, CD // 4], F32, tag="xn")
                        nc.sync.dma_start(
                            xn, xin[bt * 128:(bt + 1) * 128,
                                    qf * 512:(qf + 1) * 512])
                        xb = pin.tile([128, CD // 4], BF16, tag="xb")
                        nc.vector.tensor_copy(out=xb, in_=xn)
                        pt = psTA.tile([128, 512], BF16, tag="ptx")
                        for kk in range(4):
                            nc.tensor.transpose(
                                pt[:, kk * 128:(kk + 1) * 128],
                                xb[:, kk * 128:(kk + 1) * 128], sb_id)
                        dst = xT[:, qf * 4:(qf + 1) * 4,
                                 bt * 128:(bt + 1) * 128]
                        src = pt[:, :].rearrange("p (k b) -> p k b", k=4)
                        if (bt + qf) % 2 == 0:
                            nc.vector.tensor_copy(out=dst, in_=src)
                        else:
                            nc.scalar.copy(out=dst, in_=src)
                for nt in range(NT):
                    wst = pw.tile([128, KT, 128], BF16, tag="wst")
                    nc.sync.dma_start(
                        wst, Wt[:, nt * 128:(nt + 1) * 128].rearrange(
                            "(kt p) n -> p kt n", p=128))
                    for bc in range(2):
                        px = psA.tile([128, 512], F32, tag="px")
                        for kt in range(KT):
                            nc.tensor.matmul(
                                px, lhsT=wst[:, kt, :],
                                rhs=xT[:, kt, bc * 512:(bc + 1) * 512],
                                start=(kt == 0), stop=(kt == KT - 1))
                        dst = qkvT[t][:, nt, bc * 512:(bc + 1) * 512]
                        if with_bias:
                            nc.vector.tensor_scalar(
                                out=dst, in0=px,
                                scalar1=sb_bq[t][:, nt, 0:1],
                                scalar2=None, op0=AL.add)
                        elif (nt + bc) % 2 == 0:
                            nc.vector.tensor_copy(out=dst, in_=px)
                        else:
                            nc.scalar.copy(out=dst, in_=px)

        # ---- Phase B: attention ----
        ppk_cm = tc.tile_pool(name="ppk", bufs=1)
        ppk = ppk_cm.__enter__()
        qpk = []
        kpk = []
        vpk = []
        for i in range(2):
            qp = ppk.tile([73, 2048], BF16, name=f"qpk{i}", tag=f"qpk{i}")
            nc.sync.dma_start(qp[64:73, :], mq9[:, :])
            qpk.append(qp)
            kp = ppk.tile([73, 2048], BF16, name=f"kpk{i}", tag=f"kpk{i}")
            nc.sync.dma_start(kp[64:73, :], mk9[:, :])
            kpk.append(kp)
            vpk.append(ppk.tile([64, 2048], BF16, name=f"vpk{i}",
                                tag=f"vpk{i}"))
        vp_all = ppk.tile([128, 16, 65], BF16, name="vp_all", tag="vp_all")
        for j in range(16):
            nc.scalar.copy(out=vp_all[:, j, 64:65], in_=sb_ones)

        with tc.tile_pool(name="peT", bufs=3) as peT, \
             tc.tile_pool(name="prz", bufs=2) as prz, \
             tc.tile_pool(name="psV", bufs=2, space="PSUM") as psV, \
             tc.tile_pool(name="psS", bufs=2, space="PSUM") as psS, \
             tc.tile_pool(name="psC", bufs=2, space="PSUM") as psC, \
             tc.tile_pool(name="psR", bufs=2, space="PSUM") as psR:
            for bt in range(NB):
                for bri, (qs, ks) in enumerate((("c", "m"), ("m", "c"))):
                    idx = bt * 2 + bri
                    ib = idx % 2
                    qp, kp, vp_k = qpk[ib], kpk[ib], vpk[ib]
                    # pack q/k/v: col = j*128 + par*64 + hp*8 + b8
                    for par in range(2):
                        for pki, (pk, tsrc, base) in enumerate(
                                ((qp, qs, 0), (kp, ks, 8), (vp_k, ks, 16))):
                            src = qkvT[tsrc][
                                par * 64:(par + 1) * 64, base:base + 8,
                                bt * 128:(bt + 1) * 128].rearrange(
                                    "p hp (j b) -> p hp j b", j=16)
                            dst = pk[0:64, :].rearrange(
                                "p (j q hp b) -> p j q hp b",
                                j=16, q=2, hp=8)[:, :, par, :, :].transpose(
                                    [0, 2, 1, 3])
                            if pki == 1:
                                nc.scalar.copy(out=dst, in_=src)
                            else:
                                nc.vector.tensor_copy(out=dst, in_=src)
                    for jg in range(4):
                        psV4 = psV.tile([128, 256], BF16, tag="v4")
                        for jj in range(4):
                            j = jg * 4 + jj
                            nc.tensor.transpose(
                                psV4[:, jj * 64:(jj + 1) * 64],
                                vp_k[0:64, j * 128:(j + 1) * 128],
                                sb_id[0:64, 0:64])
                        nc.scalar.copy(
                            out=vp_all[:, jg * 4:(jg + 1) * 4, 0:64],
                            in_=psV4[:, :].rearrange("p (j d) -> p j d", j=4))
                        psS4 = psS.tile([128, 512], F32, tag="s4")
                        for jj in range(4):
                            j = jg * 4 + jj
                            nc.tensor.matmul(
                                psS4[:, jj * 128:(jj + 1) * 128],
                                lhsT=kp[:, j * 128:(j + 1) * 128],
                                rhs=qp[:, j * 128:(j + 1) * 128],
                                start=True, stop=True)
                        eT4 = peT.tile([128, 512], BF16, tag="e4")
                        nc.scalar.activation(eT4, psS4, AF.Exp, scale=0.125)
                        psC4 = psC.tile([65, 512], F32, tag="c4")
                        for jj in range(4):
                            j = jg * 4 + jj
                            nc.tensor.matmul(
                                psC4[:, jj * 128:(jj + 1) * 128],
                                lhsT=vp_all[:, j, :],
                                rhs=eT4[:, jj * 128:(jj + 1) * 128],
                                start=True, stop=True)
                        zs = prz.tile([1, 512], F32, tag="rz")
                        nc.scalar.copy(out=zs, in_=psC4[64:65, :])
                        rep_ps = psR.tile([64, 512], F32, tag="rep")
                        nc.tensor.matmul(rep_ps, lhsT=sb_o1f, rhs=zs,
                                         start=True, stop=True)
                        rep_sb = peT.tile([64, 512], F32, tag="repsb")
                        nc.vector.reciprocal_approx_fast(out=rep_sb,
                                                         in_=rep_ps)
                        for par in range(2):
                            in0 = psC4[0:64, :].rearrange(
                                "p (j q hp b) -> p j q hp b",
                                j=4, q=2, hp=8)[:, :, par, :, :]
                            in1 = rep_sb[:, :].rearrange(
                                "p (j q hp b) -> p j q hp b",
                                j=4, q=2, hp=8)[:, :, par, :, :]
                            dst = caT_all[
                                par * 64:(par + 1) * 64, idx, 0:8,
                                jg * 32:(jg + 1) * 32].rearrange(
                                    "p hp (j b) -> p hp j b", j=4).transpose(
                                        [0, 2, 1, 3])
                            nc.vector.tensor_tensor(
                                out=dst, in0=in0, in1=in1, op=AL.mult)

        ppk_cm.__exit__(None, None, None)
        pqk_cm.__exit__(None, None, None)

        # ---- LN stats (batched x4) ----
        with tc.tile_pool(name="pstt", bufs=2) as pstt, \
             tc.tile_pool(name="psqq", bufs=1) as psqq, \
             tc.tile_pool(name="psSt", bufs=2, space="PSUM") as psSt, \
             tc.tile_pool(name="psRT", bufs=2, space="PSUM") as psRT:
            sq = psqq.tile([128, 16, CT, 128], BF16, tag="sq")
            nc.vector.tensor_tensor(out=sq, in0=caT_all, in1=caT_all,
                                    op=AL.mult)
            for g4 in range(4):
                mps = psSt.tile([1, 512], F32, tag="m4")
                sps = psSt.tile([1, 512], F32, tag="sg4")
                for i in range(4):
                    idx = g4 * 4 + i
                    for ct in range(CT):
                        nc.tensor.matmul(
                            mps[0:1, i * 128:(i + 1) * 128], lhsT=sb_ones,
                            rhs=caT_all[:, idx, ct, :],
                            start=(ct == 0), stop=(ct == CT - 1))
                    for ct in range(CT):
                        nc.tensor.matmul(
                            sps[0:1, i * 128:(i + 1) * 128], lhsT=sb_ones,
                            rhs=sq[:, idx, ct, :],
                            start=(ct == 0), stop=(ct == CT - 1))
                murow = pstt.tile([1, 512], F32, tag="murow")
                nc.vector.tensor_scalar(
                    out=murow, in0=mps, scalar1=1.0 / HID, scalar2=None,
                    op0=AL.mult)
                nc.vector.tensor_copy(
                    out=mu_all[:, g4 * 512:(g4 + 1) * 512], in_=murow)
                mu2 = pstt.tile([1, 512], F32, tag="mu2")
                nc.vector.tensor_tensor(out=mu2, in0=murow, in1=murow,
                                        op=AL.mult)
                vvr = pstt.tile([1, 512], F32, tag="vvr")
                nc.vector.tensor_scalar(
                    out=vvr, in0=sps, scalar1=1.0 / HID, scalar2=EPS,
                    op0=AL.mult, op1=AL.add)
                vv2 = pstt.tile([1, 512], F32, tag="vv2")
                nc.vector.tensor_tensor(out=vv2, in0=vvr, in1=mu2,
                                        op=AL.subtract)
                rt = psRT.tile([128, 4], F32, tag="rt")
                for i in range(4):
                    nc.tensor.transpose(
                        rt[:, i:i + 1], vv2[0:1, i * 128:(i + 1) * 128],
                        sb_o1f[0:1, 0:1])
                sdc = pstt.tile([128, 4], F32, tag="sdc")
                nc.scalar.activation(sdc, rt, AF.Sqrt)
                nc.vector.reciprocal(r_all[:, g4 * 4:(g4 + 1) * 4], sdc)

        # ---- Phase C: projection + residual ----
        with tc.tile_pool(name="pwg", bufs=2) as pwg, \
             tc.tile_pool(name="pxr", bufs=4) as pxr, \
             tc.tile_pool(name="pot", bufs=4) as pot, \
             tc.tile_pool(name="psP", bufs=2, space="PSUM") as psP:
            sb_v = {}
            if with_bias:
                sb_or = consts.tile([1, 128], F32)
                nc.sync.dma_start(sb_or, onesr_f[:, :])
                for t, vv in (("c", v_c), ("m", v_m)):
                    row = consts.tile([1, CD], F32, name=f"vr_{t}",
                                      tag=f"vr_{t}")
                    nc.sync.dma_start(row, vv[:, :])
                    sb_v[t] = consts.tile([128, CD], F32, name=f"vb_{t}",
                                          tag=f"vb_{t}")
                    for ch in range(NCHP):
                        vps = psP.tile([128, 512], F32, tag="px")
                        nc.tensor.matmul(
                            vps, lhsT=sb_or,
                            rhs=row[0:1, ch * 512:(ch + 1) * 512],
                            start=True, stop=True)
                        nc.scalar.copy(
                            out=sb_v[t][:, ch * 512:(ch + 1) * 512], in_=vps)
            for nch in range(NCHP):
                wg = {}
                for t, Wgt in (("c", Wg_c), ("m", Wg_m)):
                    w = pwg.tile([128, CT, 512], BF16, tag=f"wg_{t}")
                    nc.sync.dma_start(
                        w, Wgt[:, nch * 512:(nch + 1) * 512].rearrange(
                            "(ct p) n -> p ct n", p=128))
                    wg[t] = w
                for bt in range(NB):
                    for bri, qs in enumerate(("c", "m")):
                        idx = bt * 2 + bri
                        xin = x_c if qs == "c" else x_m
                        xres = pxr.tile([128, 512], F32, tag="xr")
                        nc.sync.dma_start(
                            xres, xin[bt * 128:(bt + 1) * 128,
                                      nch * 512:(nch + 1) * 512])
                        px = psP.tile([128, 512], F32, tag="px")
                        for ct in range(CT):
                            nc.tensor.matmul(
                                px, lhsT=caT_all[:, idx, ct, :],
                                rhs=wg[qs][:, ct, :],
                                start=(ct == 0), stop=False)
                        nc.tensor.matmul(
                            px, lhsT=mu_all[:, idx * 128:(idx + 1) * 128],
                            rhs=ung[qs][:, nch * 512:(nch + 1) * 512],
                            start=False, stop=True)
                        t1 = pot.tile([128, 512], F32, tag="t1")
                        nc.scalar.activation(t1, px, AF.Copy,
                                             scale=r_all[:, idx:idx + 1])
                        ot = pot.tile([128, 512], F32, tag="ot")
                        eng = nc.vector if idx % 2 == 0 else nc.gpsimd
                        eng.tensor_tensor(out=ot, in0=t1, in1=xres, op=AL.add)
                        if with_bias:
                            o2 = pot.tile([128, 512], F32, tag="o2")
                            nc.vector.tensor_tensor(
                                out=o2, in0=ot,
                                in1=sb_v[qs][:, nch * 512:(nch + 1) * 512],
                                op=AL.add)
                            ot = o2
                        outt = out_c if qs == "c" else out_m
                        nc.sync.dma_start(
                            outt[bt * 128:(bt + 1) * 128,
                                 nch * 512:(nch + 1) * 512], ot)
    return nc


_NC = {}


def _get_nc(with_bias):
    if with_bias not in _NC:
        nc = build_nc(with_bias=with_bias)
        if not nc.is_finalized():
            nc.finalize()
        _NC[with_bias] = nc
    return _NC[with_bias]


def _host_prep(inputs):
    f32 = np.float32
    bf = ml_dtypes.bfloat16
    g = {k: np.asarray(v) for k, v in inputs.items()}
    # device ca layout: c_dev = hp*128 + par*64 + d, head h = 2*hp + par
    cdev = np.arange(HID)
    hp_t = cdev // 128
    par_t = (cdev % 128) // 64
    d_t = cdev % 64
    h_t = 2 * hp_t + par_t
    pr = d_t * H + h_t                   # ref col for each c_dev row
    consts = {}
    for t, (Wp, bp, g1, be1) in (
            ("c", ("W_cproj", "b_cproj", "g1", "be1")),
            ("m", ("W_mproj", "b_mproj", "g2", "be2"))):
        W = np.asarray(g[Wp], f32)[pr, :]          # [HID, CD] permuted
        g1d = np.asarray(g[g1], f32)[pr]
        be1d = np.asarray(g[be1], f32)[pr]
        consts[f"Wg_{t}"] = np.ascontiguousarray(
            (g1d[:, None] * W)).astype(bf)
        consts[f"v_{t}"] = (be1d @ W + np.asarray(g[bp], f32)).reshape(1, CD)\
            .astype(f32)
        consts[f"un_{t}"] = (-(g1d[:, None] * W).sum(0)).reshape(1, CD)\
            .astype(f32)
    consts["Wq_c"] = np.asarray(g["W_cqkv"], f32).astype(bf)
    consts["Wq_m"] = np.asarray(g["W_mqkv"], f32).astype(bf)
    consts["bq_c"] = np.asarray(g["b_cqkv"], f32).reshape(3 * HID, 1)
    consts["bq_m"] = np.asarray(g["b_mqkv"], f32).reshape(3 * HID, 1)
    # mask rows: col within a 128-col j-tile: c = par*64 + hp*8 + b8
    s8 = np.float32(bf(np.sqrt(800.0)))
    c = np.arange(128)
    b8 = c % 8
    mq = np.zeros((9, 128), f32)
    mk = np.zeros((9, 128), f32)
    for r in range(8):
        mq[r] = np.where(b8 == r, s8, 0.0)
        mk[r] = np.where(b8 == r, s8, 0.0)
    mq[8] = -s8
    mk[8] = s8
    consts["mq9"] = np.tile(mq, (1, 16)).astype(bf)
    consts["mk9"] = np.tile(mk, (1, 16)).astype(bf)
    consts["identb"] = np.eye(128).astype(bf)
    consts["ones_bf"] = np.ones((128, 1)).astype(bf)
    consts["ones1f"] = np.ones((1, 64)).astype(f32)
    consts["onesr_f"] = np.ones((1, 128)).astype(f32)
    return g, consts


def kernel(**inputs):
    g, consts = _host_prep(inputs)
    xc = np.ascontiguousarray(np.asarray(g["cnn_out"], np.float32))
    xm = np.ascontiguousarray(np.asarray(g["mlp_out"], np.float32))
    wb = (np.abs(consts["bq_c"]).max() > 0 or np.abs(consts["bq_m"]).max() > 0
          or np.abs(consts["v_c"]).max() > 0 or np.abs(consts["v_m"]).max() > 0)
    nc = _get_nc(bool(wb))
    in_maps = []
    for i in range(NCORES):
        m = dict(consts)
        m["x_c"] = xc[i * BS:(i + 1) * BS]
        m["x_m"] = xm[i * BS:(i + 1) * BS]
        in_maps.append(m)
    res = run_bass_kernel_spmd(nc, in_maps, list(range(NCORES))).results
    out_c = np.concatenate([np.asarray(res[i]["out_c"]) for i in range(NCORES)], 0)
    out_m = np.concatenate([np.asarray(res[i]["out_m"]) for i in range(NCORES)], 0)
    return (out_c.astype(np.float32), out_m.astype(np.float32))
